# revision 66
# baseline (speedup 1.0000x reference)
"""Trainium2 Bass kernel for nn_ConvBlock (SepGconv + LayerNorm + GELU MLP).

Computes, for full inputs:
    a   = einsum('bsc,brsd,dc->brc', x, kernel_basis, kernel_W) + conv_bias
    a   = LayerNorm(a) * ln_scale + ln_bias          (over channels, eps=1e-6)
    out = gelu_tanh(a @ W1 + b1) @ W2 + b2

Shapes: B=2, N=1024 (R=S=N), H=64, D=32, WF=4.

Sharding: the (B*R)=2048 output rows split into 8 contiguous shards of 256
rows, one per NeuronCore. Each core reads its kernel_basis shard once
(memory-bound), contracts over all S on-chip, and runs the LN/MLP tail
locally. x / weights are replicated.

Precision: the correctness gate is rel_err < 2e-2 (fro), so everything
off the fp32 PSUM accumulators runs in bf16 (measured 5.5e-3 fro on HW) —
16.8 MB/core of kernel_basis traffic, one matmul per s-chunk, and bf16
operands for every tail matmul (fp32 matmuls lower to LOW+HIGH
instruction pairs with ~300ns fp32 weight loads).

DMA strategy: the whole kernel_basis shard streams on the SYNC HWDGE
queue alone as 8 pair-granular 2 MB transfers — a single queue sustains
the full per-core HBM rate (~420 GB/s), while a second queue only adds
slow-ramp and arbitration dips, and >8 concurrent DMA objects serialize
later issues behind the Tile framework's 8 completion-sem lanes. The tiny
x tile (which gates every matmul as the stationary operand) goes first;
all other constants ride the scalar queue in two packed transfers.
Outputs use the idle gpsimd SWDGE queue (last quarter: sync, lower
completion latency).

PE strategy: H=64 means a matmul only occupies half the 128-wide PE
array. j-blocks are processed in PAIRS via column-group tiling: block A's
matmuls land in PE columns 0-63 (PSUM partitions 0-63), block B's in
columns 64-127 (partitions 64-127); col-tiled matmuls execute
CONCURRENTLY (measured 53ns B-after-A start), so a pair costs ~8 matmul
slots instead of 16 — without this the per-matmul (398+N)/2.4 isolated
cost makes the PE the pipeline pacer. A memset-dummy warmup keeps HAM at
2.4 GHz before the stream arrives.

Everything downstream lives in the split-partition layout: psum[c + 64*h,
(r^,d)] holds channels c of row-half h. The DVE drain (multiply by W
broadcast + segmented reduce over d) runs on all 128 partitions, writing
aT2[c + 64*h, 16*p + i] = a[row 32p + 16h + i, c]. LayerNorm stats are
per-column-per-half sums via a [128,2] bf16 selector matmul; rsqrt is a
DVE-only quake seed + 1 Newton step (keeps ScalarE's table pinned on
gelu); the broadcast back is a K=2 matmul with the selector transposed.
The MLP contracts each row-half with half-zeroed W1 copies (SBUF operands
at base partition 64 miscompile), and the tail runs in 4 row-quarters,
each split into SIX pipeline stages emitted one j-pair apart so no
instruction ever reaches an in-order engine-queue head before its
cross-engine dependency has completed — a single waiting tail op convoys
every later pair matmul behind it.
"""

import os

import numpy as np

import concourse.bass as bass
import concourse.tile as tile
from concourse import mybir
from concourse.bass_utils import run_bass_kernel_spmd


def _ensure_axon_hooks():
    """bass_utils imports antenv.axon_hooks when trace=True under axon; some
    images ship antenv without that module. Register a functional stand-in
    (driving NTFF capture via libaxon_pjrt.so) so tracing works, degrading
    to hook=None (no trace, run still works) if the .so is unavailable."""
    import sys
    import types

    try:
        import antenv.axon_hooks  # noqa: F401

        return
    except ImportError:
        pass
    try:
        import antenv
    except ImportError:
        antenv = types.ModuleType("antenv")
        sys.modules["antenv"] = antenv

    mod = types.ModuleType("antenv.axon_hooks")
    mod._hook = None

    def set_axon_ntff_profile_hook(h):
        mod._hook = h

    def get_axon_ntff_profile_hook():
        if mod._hook is None:
            try:
                from trn_agent_boot.trn_boot import _ntff_profile_via_ctypes

                so_path = "/opt/axon/libaxon_pjrt.so"
                if os.path.exists(so_path):
                    mod._hook = _ntff_profile_via_ctypes(so_path)
            except Exception:
                mod._hook = None
        return mod._hook

    mod.set_axon_ntff_profile_hook = set_axon_ntff_profile_hook
    mod.get_axon_ntff_profile_hook = get_axon_ntff_profile_hook
    sys.modules["antenv.axon_hooks"] = mod
    antenv.axon_hooks = mod


try:
    _ensure_axon_hooks()
except Exception:
    pass

F32 = mybir.dt.float32
BF16 = mybir.dt.bfloat16

B, N, H, D, WF = 2, 1024, 64, 32, 4
NCORES = 8
ROWS_PER_CORE = (B * N) // NCORES  # 256
RB = 16  # rows per j-block
N_JBLK = ROWS_PER_CORE // RB  # 16
N_PAIR = N_JBLK // 2  # 8
N_KCHUNK = N // 128  # 8 s-chunks of 128
FH = WF * H  # 256
LN_EPS = 1e-6

_NC_CACHE = None
LAST_EXEC_NS = None


def _build_nc(split_waits=True):
    nc = bass.Bass(target_bir_lowering=False)

    kbh = nc.dram_tensor(
        "kbh", [N_PAIR, 128, 2, N_KCHUNK, RB, D], BF16, kind="ExternalInput"
    )
    # constants ride in TWO packed DMAs (one bf16, one f32): the Tile
    # framework has only 8 DMA completion-sem lanes, and >8 in-flight DMAs
    # serialize later issues behind lane-reuse waits. All tail-matmul
    # operands are bf16 — fp32 matmuls lower to LOW+HIGH instruction pairs
    # with ~300ns fp32 weight loads, doubling tail latency.
    xcp = nc.dram_tensor("xcp", [128, N_KCHUNK, H], BF16, kind="ExternalInput")
    bpk = nc.dram_tensor("bpk", [128, 642], BF16, kind="ExternalInput")
    pack = nc.dram_tensor("pack", [128, 645], F32, kind="ExternalInput")
    sel2T = nc.dram_tensor("sel2T", [2, 128], BF16, kind="ExternalInput")
    # out[q, u, h, c] = row 64q + 32*(u//16) + 16h + (u%16), channel c
    # (host reorders; keeps the DMA a plain shape-matched copy)
    out = nc.dram_tensor("out", [4, 32, 2, H], F32, kind="ExternalOutput")

    with tile.TileContext(nc) as tc:
        with (
            tc.tile_pool(name="consts", bufs=1) as consts,
            tc.tile_pool(name="kbhp", bufs=6) as kbh_pool,
            tc.tile_pool(name="mwp", bufs=3) as mw_pool,
            tc.tile_pool(name="work", bufs=3) as work,
            tc.tile_pool(name="pmain", bufs=3, space="PSUM") as pmain,
            tc.tile_pool(name="ptail", bufs=1, space="PSUM") as ptail,
            tc.tile_pool(name="pwarm", bufs=1, space="PSUM") as pwarm,
        ):
            # ---- PE warm-up on a memset dummy tile, independent of any DMA:
            # HAM needs ~3.4us of sustained PE activity to unthrottle
            # 1.2 -> 2.4 GHz; a cold PE falls behind the DMA stream ----
            dummy = consts.tile([128, RB * D], BF16)
            nc.vector.memset(dummy, 0.25)
            ps_warm = pwarm.tile([128, RB * D], F32, name="ps_warm", tag="ps_warm")
            for w in range(18):
                nc.tensor.matmul(
                    ps_warm[0:H, :],
                    lhsT=dummy[:, 0:H],
                    rhs=dummy,
                    start=True,
                    stop=True,
                )

            # ---- packed consts ride the scalar queue; the sync queue is
            # reserved for the kernel_basis stream (a single HWDGE queue
            # sustains the full per-core HBM rate, and a second queue only
            # adds slow-ramp and arbitration dips) ----
            xc_sb = consts.tile([128, N_KCHUNK, H], BF16)
            nc.scalar.dma_start(out=xc_sb, in_=xcp[:, :, :])
            bpk_sb = consts.tile([128, 642], BF16)
            nc.scalar.dma_start(out=bpk_sb, in_=bpk[:, :])
            sel_sb = bpk_sb[:, 0:2]
            w1_sb = bpk_sb[:, 2:514]
            w2_sb = bpk_sb[:, 514:642]
            pack_sb = consts.tile([128, 645], F32)
            nc.scalar.dma_start(out=pack_sb, in_=pack[:, :])
            wb_sb = pack_sb[:, 0:512]
            cb_sb = pack_sb[:, 512:513]
            lns_sb = pack_sb[:, 513:514]
            lnb_sb = pack_sb[:, 514:515]
            b1_sb = pack_sb[:, 515:517]
            b2_sb = pack_sb[0:64, 517:645]
            selT_sb = consts.tile([2, 128], BF16)
            nc.scalar.dma_start(out=selT_sb, in_=sel2T[:, :])

            # kb pair-tiles (2 MB each, 8 transfers total) alternate between
            # the sync and scalar HWDGE queues; a whole pair arrives in one
            # DMA so the B-series can never block mid-pair.
            kb_tiles = {}

            def kb_dma(p):
                t = kbh_pool.tile(
                    [128, 2, N_KCHUNK, RB, D], BF16, name=f"kbp{p}", tag="kbp"
                )
                if p == N_PAIR - 1:
                    for jj in (0, 1):
                        nc.sync.dma_start(
                            out=t[:, jj, :, :, :], in_=kbh[p, :, jj, :, :, :]
                        )
                else:
                    nc.sync.dma_start(out=t, in_=kbh[p, :, :, :, :, :])
                kb_tiles[p] = t

            for p0 in range(5):
                kb_dma(p0)

            # aT2[c + 64h, 16p + i] = a[row 32p + 16h + i, channel c]
            aT2 = consts.tile([128, N_PAIR * RB], F32)

            # ---- tail pieces, per quarter of rows (64 each, = aT2 cols
            # 32q..32q+32), emission staggered through the pair loop ----
            QC = 2 * RB  # 32 aT2 columns per quarter
            state = {}

            def t_stacked(q):
                sl = slice(QC * q, QC * (q + 1))
                st = work.tile([128, 2 * QC], BF16, name=f"stacked{q}", tag="stacked")
                nc.vector.tensor_scalar(
                    out=st[:, 0:QC], in0=aT2[:, sl], scalar1=cb_sb,
                    scalar2=None, op0=mybir.AluOpType.add,
                )
                nc.vector.tensor_mul(st[:, QC : 2 * QC], st[:, 0:QC], st[:, 0:QC])
                state[("st", q)] = st

            def t_stats_mm(q):
                st = state[("st", q)]
                ps_s = ptail.tile([2, 2 * QC], F32, name=f"ps_s{q}", tag="ps_s", bufs=1)
                nc.tensor.matmul(ps_s, lhsT=sel_sb, rhs=st, start=True, stop=True)
                state[("ps_s", q)] = ps_s

            def t_stats_dve(q):
                ps_s = state[("ps_s", q)]
                m = work.tile([2, 2 * QC], F32, name=f"m{q}", tag="m")
                nc.vector.tensor_scalar(
                    out=m, in0=ps_s, scalar1=1.0 / H, scalar2=None,
                    op0=mybir.AluOpType.mult,
                )
                var = work.tile([2, QC], F32, name=f"var{q}", tag="var")
                nc.vector.tensor_mul(var, m[:, 0:QC], m[:, 0:QC])
                qt = work.tile([2, QC], F32, name=f"qt{q}", tag="qt")
                nc.vector.scalar_tensor_tensor(
                    out=qt, in0=m[:, QC : 2 * QC], scalar=LN_EPS, in1=var,
                    op0=mybir.AluOpType.add, op1=mybir.AluOpType.subtract,
                )
                # rsqrt on DVE only (keeps ScalarE's table pinned on gelu):
                # quake seed via int<->float value casts, then 1 Newton step
                # (~1.7e-3 rel err, well inside the 2e-2 budget).
                yi = work.tile([2, QC], mybir.dt.int32, name=f"yi{q}", tag="yi")
                nc.vector.tensor_scalar(
                    out=yi, in0=qt.bitcast(mybir.dt.int32),
                    scalar1=-0.5, scalar2=float(0x5F3759DF),
                    op0=mybir.AluOpType.mult, op1=mybir.AluOpType.add,
                )
                y = yi.bitcast(F32)
                t1 = work.tile([2, QC], F32, name=f"t1_{q}", tag="t1")
                nc.vector.tensor_mul(t1, y, y)
                nc.vector.tensor_mul(t1, t1, qt)
                nc.vector.tensor_scalar(
                    out=t1, in0=t1, scalar1=-0.5, scalar2=1.5,
                    op0=mybir.AluOpType.mult, op1=mybir.AluOpType.add,
                )
                rp = work.tile([2, 2 * QC], BF16, name=f"rp{q}", tag="rp")
                nc.vector.tensor_mul(rp[:, 0:QC], y, t1)
                nc.vector.tensor_mul(rp[:, QC : 2 * QC], m[:, 0:QC], rp[:, 0:QC])
                state[("rp", q)] = rp

            def t_bc_mm(q):
                rp = state[("rp", q)]
                ps_bc = ptail.tile(
                    [128, 2 * QC], F32, name=f"ps_bc{q}", tag="ps_bc", bufs=1
                )
                nc.tensor.matmul(ps_bc, lhsT=selT_sb, rhs=rp, start=True, stop=True)
                state[("ps_bc", q)] = ps_bc

            def t_bc_dve(q):
                ps_bc = state[("ps_bc", q)]
                st = state[("st", q)]
                aln = work.tile([128, QC], BF16, name=f"aln{q}", tag="aln")
                nc.vector.tensor_mul(aln, st[:, 0:QC], ps_bc[:, 0:QC])
                nc.vector.tensor_sub(aln, aln, ps_bc[:, QC : 2 * QC])
                state[("aln", q)] = aln

            def t_mlp_a(q):
                aln = state[("aln", q)]
                hT = work.tile([128, 2, 2 * QC], BF16, name=f"hT{q}", tag="hT")
                state[("hT", q)] = hT
                for fh in range(2):
                    ph = ptail.tile(
                        [128, 2, QC], F32, name=f"ph{q}_{fh}", tag="ph", bufs=1
                    )
                    # per-half contraction via half-zeroed W1 copies: operands
                    # at SBUF base partition 64 miscompile (HW crash), so both
                    # matmuls contract all 128 partitions with the other
                    # half's weights zeroed.
                    for h in range(2):
                        nc.tensor.matmul(
                            ph[:, h, :],
                            lhsT=w1_sb[:, FH * h + 128 * fh - 0 : FH * h + 128 * (fh + 1)],
                            rhs=aln,
                            start=True,
                            stop=True,
                            skip_group_check=(h == 1),
                        )
                    nc.scalar.activation(
                        out=hT[:, fh, :],
                        in_=ph.rearrange("p a b -> p (a b)"),
                        func=mybir.ActivationFunctionType.Gelu_apprx_tanh,
                        bias=b1_sb[:, fh : fh + 1],
                        scale=1.0,
                    )
            def t_mlp_b(q):
                hT = state[("hT", q)]
                po = ptail.tile([32, 2, H], F32, name=f"po{q}", tag="po", bufs=1)
                for h in range(2):
                    for fh in range(2):
                        nc.tensor.matmul(
                            po[:, h, :],
                            lhsT=hT[:, fh, QC * h : QC * (h + 1)],
                            rhs=w2_sb[:, 64 * fh : 64 * (fh + 1)],
                            start=(fh == 0),
                            stop=(fh == 1),
                            skip_group_check=(h == 1),
                        )
                o_sb = work.tile([32, 2, H], F32, name=f"o_sb{q}", tag="o_sb")
                nc.vector.tensor_add(
                    o_sb.rearrange("p a b -> p (a b)"),
                    po.rearrange("p a b -> p (a b)"),
                    b2_sb[0:32, :],
                )
                # early quarters ride the idle gpsimd SWDGE queue (a kb-queue
                # dma_start would make later kb issues wait behind o_sb); the
                # last quarter uses sync HWDGE — the kb stream is done and
                # HWDGE completes ~1us faster than SWDGE.
                if q == 3:
                    nc.sync.dma_start(out=out[q, :, :, :], in_=o_sb)
                else:
                    nc.gpsimd.dma_start(out=out[q, :, :, :], in_=o_sb)

            def t_stacked23a():
                # pairs 4-6 portion (48 of 64 cols): deps are drains p4-p6,
                # so this runs in the DVE idle window before drain(p7)
                st = work.tile([128, 4 * QC], BF16, name="stacked23", tag="stacked")
                nc.vector.tensor_scalar(
                    out=st[:, 0:48], in0=aT2[:, 2 * QC : 2 * QC + 48],
                    scalar1=cb_sb, scalar2=None, op0=mybir.AluOpType.add,
                )
                nc.vector.tensor_mul(
                    st[:, 2 * QC : 2 * QC + 48], st[:, 0:48], st[:, 0:48]
                )
                state["st23"] = st
                ps_s = ptail.tile([2, 4 * QC], F32, name="ps_s23", tag="ps_s", bufs=1)
                nc.tensor.matmul(
                    ps_s[:, 0:48], lhsT=sel_sb, rhs=st[:, 0:48],
                    start=True, stop=True,
                )
                nc.tensor.matmul(
                    ps_s[:, 2 * QC : 2 * QC + 48], lhsT=sel_sb,
                    rhs=st[:, 2 * QC : 2 * QC + 48],
                    start=True, stop=True, skip_group_check=True,
                )
                state["ps_s23"] = ps_s

            def t_stacked23b():
                # pair-7 remainder (16 cols): the only stacked work on the
                # post-drain(p7) critical path
                st = state["st23"]
                nc.vector.tensor_scalar(
                    out=st[:, 48 : 2 * QC], in0=aT2[:, 2 * QC + 48 : 4 * QC],
                    scalar1=cb_sb, scalar2=None, op0=mybir.AluOpType.add,
                )
                nc.vector.tensor_mul(
                    st[:, 2 * QC + 48 : 4 * QC], st[:, 48 : 2 * QC],
                    st[:, 48 : 2 * QC],
                )

            def t_stats_mm23b():
                st = state["st23"]
                ps_s = state["ps_s23"]
                nc.tensor.matmul(
                    ps_s[:, 48 : 2 * QC], lhsT=sel_sb, rhs=st[:, 48 : 2 * QC],
                    start=True, stop=True, skip_group_check=True,
                )
                nc.tensor.matmul(
                    ps_s[:, 2 * QC + 48 : 4 * QC], lhsT=sel_sb,
                    rhs=st[:, 2 * QC + 48 : 4 * QC],
                    start=True, stop=True, skip_group_check=True,
                )

            def t_stats_dve23():
                ps_s = state["ps_s23"]
                W2Q = 2 * QC
                m = work.tile([2, 2 * W2Q], F32, name="m23", tag="m")
                nc.vector.tensor_scalar(
                    out=m, in0=ps_s, scalar1=1.0 / H, scalar2=None,
                    op0=mybir.AluOpType.mult,
                )
                var = work.tile([2, W2Q], F32, name="var23", tag="var")
                nc.vector.tensor_mul(var, m[:, 0:W2Q], m[:, 0:W2Q])
                qt = work.tile([2, W2Q], F32, name="qt23", tag="qt")
                nc.vector.scalar_tensor_tensor(
                    out=qt, in0=m[:, W2Q : 2 * W2Q], scalar=LN_EPS, in1=var,
                    op0=mybir.AluOpType.add, op1=mybir.AluOpType.subtract,
                )
                yi = work.tile([2, W2Q], mybir.dt.int32, name="yi23", tag="yi")
                nc.vector.tensor_scalar(
                    out=yi, in0=qt.bitcast(mybir.dt.int32),
                    scalar1=-0.5, scalar2=float(0x5F3759DF),
                    op0=mybir.AluOpType.mult, op1=mybir.AluOpType.add,
                )
                y = yi.bitcast(F32)
                t1 = work.tile([2, W2Q], F32, name="t1_23", tag="t1")
                nc.vector.tensor_mul(t1, y, y)
                nc.vector.tensor_mul(t1, t1, qt)
                nc.vector.tensor_scalar(
                    out=t1, in0=t1, scalar1=-0.5, scalar2=1.5,
                    op0=mybir.AluOpType.mult, op1=mybir.AluOpType.add,
                )
                rp = work.tile([2, 2 * W2Q], BF16, name="rp23", tag="rp")
                nc.vector.tensor_mul(rp[:, 0:W2Q], y, t1)
                nc.vector.tensor_mul(rp[:, W2Q : 2 * W2Q], m[:, 0:W2Q], rp[:, 0:W2Q])
                state["rp23"] = rp

            def t_bc23():
                rp = state["rp23"]
                st = state["st23"]
                W2Q = 2 * QC
                ps_bc = ptail.tile(
                    [128, 2 * W2Q], F32, name="ps_bc23", tag="ps_bc", bufs=1
                )
                nc.tensor.matmul(ps_bc, lhsT=selT_sb, rhs=rp, start=True, stop=True)
                aln = work.tile([128, W2Q], BF16, name="aln23", tag="aln")
                nc.vector.tensor_mul(aln, st[:, 0:W2Q], ps_bc[:, 0:W2Q])
                nc.vector.tensor_sub(aln, aln, ps_bc[:, W2Q : 2 * W2Q])
                state["aln23"] = aln

            def t_mlp23():
                aln = state["aln23"]
                W2Q = 2 * QC
                hT = work.tile([128, 2, 2 * W2Q], BF16, name="hT23", tag="hT")
                for fh in range(2):
                    ph = ptail.tile(
                        [128, 2, W2Q], F32, name=f"ph23_{fh}", tag="ph", bufs=1
                    )
                    for h in range(2):
                        nc.tensor.matmul(
                            ph[:, h, :],
                            lhsT=w1_sb[:, FH * h + 128 * fh : FH * h + 128 * (fh + 1)],
                            rhs=aln,
                            start=True,
                            stop=True,
                            skip_group_check=(h == 1),
                        )
                    nc.scalar.activation(
                        out=hT[:, fh, :],
                        in_=ph.rearrange("p a b -> p (a b)"),
                        func=mybir.ActivationFunctionType.Gelu_apprx_tanh,
                        bias=b1_sb[:, fh : fh + 1],
                        scale=1.0,
                    )
                po = ptail.tile([64, 2, H], F32, name="po23", tag="po", bufs=1)
                for h in range(2):
                    for fh in range(2):
                        nc.tensor.matmul(
                            po[:, h, :],
                            lhsT=hT[:, fh, W2Q * h : W2Q * (h + 1)],
                            rhs=w2_sb[:, 64 * fh : 64 * (fh + 1)],
                            start=(fh == 0),
                            stop=(fh == 1),
                            skip_group_check=(h == 1),
                        )
                o_sb = work.tile([64, 2, H], F32, name="o_sb23", tag="o_sb")
                nc.vector.tensor_add(
                    o_sb.rearrange("p a b -> p (a b)"),
                    po.rearrange("p a b -> p (a b)"),
                    b2_sb,
                )
                nc.sync.dma_start(out=out[2, :, :, :], in_=o_sb[0:32, :, :])
                nc.sync.dma_start(out=out[3, :, :, :], in_=o_sb[32:64, :, :])

            sched = {
                2: [lambda: t_stacked(0)],
                3: [lambda: t_stats_mm(0)],
                4: [lambda: t_stats_dve(0), lambda: t_stacked(1)],
                5: [lambda: t_bc_mm(0), lambda: t_stats_mm(1)],
                6: [
                    lambda: t_bc_dve(0), lambda: t_mlp_a(0),
                    lambda: t_stats_dve(1),
                ],
                7: [
                    lambda: t_mlp_b(0), lambda: t_bc_mm(1),
                    lambda: t_stacked23a,
                ][0:2] + [t_stacked23a],
            }

            # ---- main contraction: j-block pairs, col-group tiled.
            # Block A (even j) -> PE cols 0-63 / PSUM partitions 0-63,
            # block B (odd j) -> cols 64-127 / partitions 64-127; the two
            # matmul streams execute concurrently in the array. ----
            for p in range(N_PAIR):
                if p not in kb_tiles:
                    kb_dma(p)
                pt = kb_tiles.pop(p)
                if p + 5 < N_PAIR:
                    kb_dma(p + 5)
                ps = pmain.tile([128, RB * D], F32, name="ps", tag="ps")
                for k in range(N_KCHUNK):
                    for jj in (0, 1):
                        # the sim's psum group bookkeeping is partition-base
                        # blind, so only the A series (partitions 0-63) does
                        # the bookkeeping; B is an independent per-partition
                        # accumulation group on partitions 64-127.
                        nc.tensor.matmul(
                            ps[64 * jj : 64 * (jj + 1), :],
                            lhsT=xc_sb[:, k, :],
                            rhs=pt[:, jj, k, :, :],
                            start=(k == 0),
                            stop=(k == N_KCHUNK - 1),
                            skip_group_check=(jj == 1),
                        )
                # warm-keeper dummies: per-pair PE busy (~1.7us) leaves idle
                # gaps at HAM's 3.4us re-throttle threshold; ~0.7us of filler
                # keeps the clock at 2.4 GHz for the endgame matmuls.
                for w in range(3):
                    nc.tensor.matmul(
                        ps_warm[0:H, :], lhsT=dummy[:, 0:H], rhs=dummy,
                        start=True, stop=True,
                    )
                # tail work for earlier quarters is emitted BEFORE this pair's
                # drain: engine queues are in-order, so anything emitted after
                # the drain would wait on this pair's matmuls finishing.
                for fn in sched.get(p, ()):
                    fn()
                mw = mw_pool.tile([128, RB, D], F32)
                nc.vector.tensor_mul(
                    mw.rearrange("p a b -> p (a b)"), ps, wb_sb
                )
                nc.vector.tensor_reduce(
                    out=aT2[:, RB * p : RB * (p + 1)],
                    in_=mw,
                    axis=mybir.AxisListType.X,
                    op=mybir.AluOpType.add,
                )

            # remaining tail after the stream: finish q1, then quarters 2+3
            # as double-width combined stages (half the op count of two
            # serial chains), with the per-quarter MLP/out path unchanged
            t_bc_dve(1)
            t_mlp_a(1)
            t_stacked23b()
            t_mlp_b(1)
            t_stats_mm23b()
            t_stats_dve23()
            t_bc23()
            t_mlp23()

    if split_waits:
        _split_matmul_waits(nc)
    return nc


def _split_matmul_waits(nc):
    """This walrus build rejects engine instructions carrying more than one
    semaphore wait ("Too many sync wait commands"). Peel all but the last
    wait off onto same-engine NoOps inserted immediately before the
    instruction — NoOps execute in queue order on the same sequencer, so the
    wait semantics are unchanged."""
    f = nc.m.functions[0]
    nop_id = 0
    for blk in f.blocks:
        insts = list(blk.instructions)
        out = []
        changed = False
        for inst in insts:
            si = inst.sync_info
            if (
                si is not None
                and si.on_wait is not None
                and len(si.on_wait) > 1
                and getattr(inst, "engine", None) is not None
            ):
                waits = list(si.on_wait)
                for w in waits[:-1]:
                    nop = mybir.InstNoOp(
                        name=f"I-mmwait-{nop_id}",
                        engine=inst.engine,
                        ins=[],
                        outs=[],
                        sync_info=mybir.SyncInfo(on_wait=[w], on_update=[]),
                    )
                    nop_id += 1
                    out.append(nop)
                inst.sync_info = mybir.SyncInfo(
                    on_wait=[waits[-1]], on_update=list(si.on_update or [])
                )
                changed = True
            out.append(inst)
        if changed:
            blk.instructions = out


def _get_nc():
    global _NC_CACHE
    if _NC_CACHE is None:
        _NC_CACHE = _build_nc()
    return _NC_CACHE


def _prep_shared(kernel_W, conv_bias, ln_scale, ln_bias, W1, b1, W2, b2):
    import ml_dtypes

    # f32 pack: wb2 | (cb, lns, lnb) | b1p | b2d — offsets match pack_sb
    pk = np.zeros((128, 645), np.float32)
    # wb2[c + 64h, r^*D + d] = W[d, c] — W broadcast over both row-halves
    pk[:, 0:512] = np.tile(kernel_W.T.astype(np.float32), (2, RB))
    pk[:, 512:515] = np.tile(
        np.stack([conv_bias, ln_scale, ln_bias], axis=1), (2, 1)
    )
    pk[:, 515:517] = b1.reshape(2, 128).T
    pk[0:64, 517:645] = np.tile(b2, (64, 2))
    # bf16 pack: sel2 | w1z | w2p
    sel2 = np.zeros((128, 2), np.float32)
    sel2[0:64, 0] = 1.0
    sel2[64:128, 1] = 1.0
    W1f = ln_scale[:, None].astype(np.float32) * W1
    b1 = b1 + ln_bias.astype(np.float32) @ W1
    bq = np.zeros((128, 642), ml_dtypes.bfloat16)
    bq[:, 0:2] = sel2
    bq[0:64, 2 : 2 + FH] = W1f
    bq[64:128, 2 + FH : 2 + 2 * FH] = W1f
    bq[:, 514:642] = (
        W2.reshape(2, 128, H).transpose(1, 0, 2).reshape(128, 2 * H)
    )
    sel2T = np.ascontiguousarray(sel2.T.astype(ml_dtypes.bfloat16))
    return dict(pack=np.ascontiguousarray(pk), sel2T=sel2T, bpk=np.ascontiguousarray(bq))


def _prep_x(xb):
    # (N, H) -> (128, k, H) bf16, with s = 128*k + p
    import ml_dtypes

    xh = xb.astype(ml_dtypes.bfloat16)
    return np.ascontiguousarray(xh.reshape(N_KCHUNK, 128, H).transpose(1, 0, 2))


def _prep_kb_shard(shard):
    # shard (256, 1024, 32) bf16 -> (pair, p, jj, k, r^, d)
    import ml_dtypes

    hi = shard.astype(ml_dtypes.bfloat16)
    return np.ascontiguousarray(
        hi.reshape(N_PAIR, 2, RB, N_KCHUNK, 128, D).transpose(0, 4, 1, 3, 2, 5)
    )


def kernel(
    x,
    kernel_basis,
    kernel_W,
    conv_bias,
    ln_scale,
    ln_bias,
    W1,
    b1,
    W2,
    b2,
):
    global LAST_EXEC_NS
    x = np.ascontiguousarray(np.asarray(x, np.float32))
    kb = np.ascontiguousarray(np.asarray(kernel_basis, np.float32))
    shared = _prep_shared(
        np.asarray(kernel_W, np.float32),
        np.asarray(conv_bias, np.float32),
        np.asarray(ln_scale, np.float32),
        np.asarray(ln_bias, np.float32),
        np.asarray(W1, np.float32),
        np.asarray(b1, np.float32),
        np.asarray(W2, np.float32),
        np.asarray(b2, np.float32),
    )
    xps = [_prep_x(x[b]) for b in range(B)]

    kbf = kb.reshape(B * N, N, D)
    in_maps = []
    for c in range(NCORES):
        hi = _prep_kb_shard(kbf[c * ROWS_PER_CORE : (c + 1) * ROWS_PER_CORE])
        in_maps.append(dict(kbh=hi, xcp=xps[c // (NCORES // B)], **shared))

    nc = _get_nc()
    trace = bool(os.environ.get("KERNEL_BASS_TRACE"))
    res = run_bass_kernel_spmd(nc, in_maps, core_ids=list(range(NCORES)), trace=trace)
    LAST_EXEC_NS = res.exec_time_ns

    # out[q, u, h, c]: row = 64q + 32*(u//16) + 16h + (u%16)
    outs = np.concatenate(
        [
            res.results[c]["out"]
            .reshape(4, 2, RB, 2, H)
            .transpose(0, 1, 3, 2, 4)
            .reshape(ROWS_PER_CORE, H)
            for c in range(NCORES)
        ],
        axis=0,
    )
    return outs.reshape(B, N, H)
